# revision 68
# baseline (speedup 1.0000x reference)
"""ANOVA kernel (order 3) on 8 TRN2 NeuronCores.

Math: out[b] = sum_e e3(x[b, :, e]) where e3 is the 3rd elementary
symmetric polynomial over the field axis. Via Newton's identities:
    e3 = (p1^3 - 3*p1*p2 + 2*p3) / 6,   p_k = sum_f x^k
so the kernel is: elementwise x^2 (ScalarE/VectorE), x^3 (VectorE),
field-axis reductions on TensorE (matmuls with one-hot selector
weights, bf16), then a small fused finale.

Data parallel over batch: core c handles b in [1024*c, 1024*(c+1)).

Layout per core: tile tau covers 16 consecutive b. SBUF tile (128, 512)
bf16: partition p = b_q*32 + fp (b_q in [0,4), f-pair fp in [0,32)),
free n = j2*128 + parity*64 + e. This gives 512-byte contiguous DMA
descriptors (f-pairs): DRAM offset = p*128 + j2*16384 + parity*64 + e.

A matmul with a 32-column one-hot selector lhsT (col m one-hot at
m = 4*(tau' % 8) + b_q) accumulates each tile's f-pair sums into PSUM
rows 4*tau' + b_q of a 32-row block; 32 tiles fill a (128, 512) PSUM
tensor per stat (one fill per half of the core's batch). The finale
adds the two f-parity halves, applies Newton's formula, and reduces
over e. The first fill's finale runs mid-kernel, hidden under DMA.
"""

import sys

if "/opt/trn_rl_repo" not in sys.path:
    sys.path.insert(0, "/opt/trn_rl_repo")

import numpy as np

N_CORES = 8
B, F, E = 8192, 64, 64
B_PER_CORE = B // N_CORES  # 1024
J2 = 4                     # b-quads per tile
FD = 512                   # tile free dim = J2 * 2 * E (one PSUM bank)
TILES = B_PER_CORE // 16   # 64 (16 b per tile)
SUPER = 4                  # tiles per superblock for big ACT/DVE ops
N_SUPER = TILES // SUPER   # 16
SFD = FD * SUPER           # 2048
FILL_SUPERS = 8            # supers per PSUM fill (32 tiles = 128 rows)
DVE_SQUARE_SUPERS = {2, 6, 10, 14}  # squares computed on DVE, not ACT

_cache = {}


def _make_g() -> np.ndarray:
    """One-hot selector weights (128, 124) bf16: row k has a 1 at col
    60 + k//32. lhsT for tile tau' is g[:, 60-4*m16 : 124-4*m16] with
    m16 = tau' % 16, so lhsT[k, m] = 1 iff m == 4*m16 + k//32."""
    import ml_dtypes

    g = np.zeros((128, 124), dtype=ml_dtypes.bfloat16)
    for k in range(128):
        g[k, 60 + k // 32] = 1.0
    return g


def _build():
    import concourse.bass as bass
    import concourse.tile as tile
    from concourse import bacc, mybir

    nc = bacc.Bacc(
        "TRN2", target_bir_lowering=False, debug=False, num_devices=N_CORES
    )
    f32 = mybir.dt.float32
    bf16 = mybir.dt.bfloat16

    x_dram = nc.dram_tensor(
        "x", [B_PER_CORE, F, E], f32, kind="ExternalInput"
    ).ap()
    g_dram = nc.dram_tensor("g", [128, 124], bf16, kind="ExternalInput").ap()
    out_dram = nc.dram_tensor("out", [128, 2 * J2], f32, kind="ExternalOutput").ap()

    TILE_ELEMS = 16 * F * E  # 65536

    def x_ap(tile0: int, ntiles: int) -> bass.AP:
        ap = [[128, 128]]  # partition (b_q, fp): uniform stride 128
        if ntiles > 1:
            ap.append([TILE_ELEMS, ntiles])
        ap += [[4 * F * E, J2], [1, 2 * E]]  # j2, (parity e)
        return bass.AP(tensor=x_dram.tensor, offset=tile0 * TILE_ELEMS, ap=ap)

    with tile.TileContext(nc) as tc:
        with (
            tc.tile_pool(name="const", bufs=1) as const_pool,
            tc.tile_pool(name="xin", bufs=12) as x_pool,
            tc.tile_pool(name="xsq", bufs=6) as x2_pool,
            tc.tile_pool(name="xcu", bufs=6) as x3_pool,
            tc.tile_pool(name="acc", bufs=1, space="PSUM") as psum_pool,
            tc.tile_pool(name="tail", bufs=2) as tail_pool,
        ):
            g_sb = const_pool.tile([128, 124], bf16)
            nc.sync.dma_start(out=g_sb[:], in_=g_dram[:])
            outt = const_pool.tile([128, 2 * J2], f32)

            psums = [
                [
                    psum_pool.tile([128, FD], f32, name=f"psum_{phi}_{stat}")
                    for stat in range(3)
                ]
                for phi in range(2)
            ]

            def finale(phi: int):
                """e3 = (p1^3 - 3 p1 p2 + 2 p3)/6 summed over e, for one
                PSUM fill. Starts by summing the two f-parity halves."""
                p1t, p2t, p3t = psums[phi]
                pa = []
                for idx, pt in enumerate((p1t, p2t, p3t)):
                    v = pt[:].rearrange("p (j t e) -> p j t e", j=J2, t=2)
                    a = tail_pool.tile([128, J2, E], f32, name=f"pa{idx}")
                    # DVE can read only one PSUM operand: stage parity 0
                    # through ACT, then add parity 1 (PSUM) on DVE.
                    nc.scalar.copy(a[:], v[:, :, 0, :])
                    nc.vector.tensor_add(a[:], a[:], v[:, :, 1, :])
                    pa.append(a)
                pa1, pa2, pa3 = pa
                t1 = tail_pool.tile([128, J2 * E], f32)
                nc.scalar.square(t1[:], pa1[:])  # p1^2
                u2 = tail_pool.tile([128, J2 * E], f32)
                nc.vector.scalar_tensor_tensor(  # p1^2 - 3 p2
                    u2[:], pa2[:], -3.0, t1[:],
                    op0=mybir.AluOpType.mult, op1=mybir.AluOpType.add,
                )
                u3 = tail_pool.tile([128, J2 * E], f32)
                nc.vector.tensor_mul(u3[:], u2[:], pa1[:])  # p1^3 - 3 p1 p2
                u5 = tail_pool.tile([128, J2 * E], f32)
                nc.vector.scalar_tensor_tensor(  # + 2 p3
                    u5[:], pa3[:], 2.0, u3[:],
                    op0=mybir.AluOpType.mult, op1=mybir.AluOpType.add,
                )
                red = tail_pool.tile([128, J2], f32)
                nc.vector.reduce_sum(
                    red[:],
                    u5[:].rearrange("p (j e) -> p j e", j=J2),
                    axis=mybir.AxisListType.X,
                )
                nc.vector.tensor_scalar_mul(
                    outt[:, J2 * phi : J2 * (phi + 1)], red[:], 1.0 / 6.0
                )
                # per-fill output store: fill 0's half is hidden mid-kernel
                nc.sync.dma_start(
                    out=out_dram[:, J2 * phi : J2 * (phi + 1)],
                    in_=outt[:, J2 * phi : J2 * (phi + 1)],
                )

            for s in range(N_SUPER):
                xb = x_pool.tile([128, SFD], bf16)
                # SWDGE cast-DMA: fp32 DRAM -> bf16 SBUF. First and last
                # superblocks are split to shorten ramp and tail.
                nsplit = (
                    SUPER if s == 0 else 2 if s in (1, 2, 3) else 1
                )
                csz = SFD // nsplit
                for c in range(nsplit):
                    nc.gpsimd.dma_start(
                        out=xb[:, c * csz : (c + 1) * csz],
                        in_=x_ap(
                            s * SUPER + c * (SUPER // nsplit), SUPER // nsplit
                        ),
                    )
                x2b = x2_pool.tile([128, SFD], bf16)
                x3b = x3_pool.tile([128, SFD], bf16)
                for c in range(nsplit):
                    cs = slice(c * csz, (c + 1) * csz)
                    if s in DVE_SQUARE_SUPERS:
                        nc.vector.tensor_mul(x2b[:, cs], xb[:, cs], xb[:, cs])
                    else:
                        nc.scalar.square(x2b[:, cs], xb[:, cs])
                    nc.vector.tensor_mul(x3b[:, cs], x2b[:, cs], xb[:, cs])
                # matmuls grouped by stat to limit PSUM bank switching
                phi = s // FILL_SUPERS
                for stat, src in enumerate((xb, x2b, x3b)):
                    psum = psums[phi][stat]
                    for k in range(SUPER):
                        taup = (s % FILL_SUPERS) * SUPER + k
                        m16, cg = taup % 16, taup // 16
                        nc.tensor.matmul(
                            psum[64 * cg : 64 * cg + 64, :],
                            g_sb[:, 60 - 4 * m16 : 124 - 4 * m16],
                            src[:, k * FD : (k + 1) * FD],
                            start=m16 == 0,
                            stop=m16 == 15,
                            skip_group_check=True,
                        )
                if s % FILL_SUPERS == FILL_SUPERS - 1:
                    finale(s // FILL_SUPERS)

    nc.compile()
    return nc


def _get_nc():
    if "nc" not in _cache:
        _cache["nc"] = _build()
    return _cache["nc"]


def _unpermute(r: np.ndarray) -> np.ndarray:
    # r[4*tau' + b_q, J2*phi + j2] is the value for
    # b = 512*phi + 16*tau' + 4*j2 + b_q
    return np.transpose(r.reshape(32, 4, 2, J2), (2, 0, 3, 1)).reshape(-1)


def _run(x: np.ndarray, **kwargs):
    from concourse.bass_utils import run_bass_kernel_spmd

    nc = _get_nc()
    g = _make_g()
    shards = x.reshape(N_CORES, B_PER_CORE, F, E)
    in_maps = [
        {"x": np.ascontiguousarray(shards[c]), "g": g} for c in range(N_CORES)
    ]
    res = run_bass_kernel_spmd(nc, in_maps, core_ids=list(range(N_CORES)), **kwargs)
    out = np.concatenate(
        [_unpermute(np.asarray(res.results[c]["out"])) for c in range(N_CORES)]
    ).astype(np.float32)
    return out, res


def kernel(**inputs) -> np.ndarray:
    x = np.ascontiguousarray(np.asarray(inputs["x"], dtype=np.float32))
    assert x.shape == (B, F, E), x.shape
    out, _ = _run(x)
    return out
